# revision 1
# baseline (speedup 1.0000x reference)
"""Self-contained 8-core Trainium2 Bass kernel for the BaseGNN problem.

kernel(**inputs) -> np.ndarray [50000, 72] float32.
Strategy: degree-sorted node sharding across 8 NeuronCores; per conv layer
h' = dinv*h is allgathered as a bf16 node-major table in DRAM, edge messages
are fetched with indirect-DMA gathers (128 rows/call), aggregated per
128-dst-node block on the TensorEngine into PSUM, normalized (BN stats via
AllReduce) and activated; encoder/classifier matmuls are fused in.
"""
import numpy as np
import ml_dtypes

import jax
from jax.sharding import Mesh, PartitionSpec
from jax.experimental.shard_map import shard_map

import concourse.bacc as bacc
import concourse.tile as tile
import concourse.mybir as mybir
from concourse import bass
from concourse.bass2jax import _bass_exec_p, install_neuronx_cc_hook, partition_id_tensor

N = 50000
E = 1000000
bf16_np = ml_dtypes.bfloat16

F_IN = 16
HID = 128
N_CLS = 72
EPS = 1e-5
NC = 8
PER_CORE = 6250
BLOCKS = 49
LSHARD = BLOCKS * 128
TOT = NC * LSHARD
N_REAL = 50000
CG = 1  # chunks per gather call (multi-call indirect DMA is only safe at 128 descs)

f32 = mybir.dt.float32
bf16 = mybir.dt.bfloat16
i32 = mybir.dt.int32


def build(Db, nchunks, debug=False):
    chunk_base = np.concatenate([[0], np.cumsum(Db)]).astype(int)
    nc = bacc.Bacc("TRN2", target_bir_lowering=False, debug=False,
                   enable_asserts=True, num_devices=NC)

    # ---- inputs ----
    xT_d = nc.dram_tensor("xT", [F_IN, LSHARD], f32, kind="ExternalInput")
    idx_d = nc.dram_tensor("idx", [128, nchunks], i32, kind="ExternalInput")
    dinv_d = nc.dram_tensor("dinvbc", [128, LSHARD], f32, kind="ExternalInput")
    wenc_d = nc.dram_tensor("W_enc", [F_IN, HID], f32, kind="ExternalInput")
    benc_d = nc.dram_tensor("b_enc", [HID, 1], f32, kind="ExternalInput")
    wc_d = nc.dram_tensor("W_conv", [3, HID, HID], bf16, kind="ExternalInput")
    bng_d = nc.dram_tensor("bn_g", [3, HID, 1], f32, kind="ExternalInput")
    bnb_d = nc.dram_tensor("bn_b", [3, HID, 1], f32, kind="ExternalInput")
    wc1_d = nc.dram_tensor("W_cls1", [HID, 64], bf16, kind="ExternalInput")
    bc1_d = nc.dram_tensor("b_cls1", [64, 1], f32, kind="ExternalInput")
    wc2_d = nc.dram_tensor("W_cls2", [64, N_CLS], bf16, kind="ExternalInput")
    bc2_d = nc.dram_tensor("b_cls2", [N_CLS, 1], f32, kind="ExternalInput")
    ident_d = nc.dram_tensor("ident", [128, 128], bf16, kind="ExternalInput")
    out_d = nc.dram_tensor("outT", [N_CLS, LSHARD], f32, kind="ExternalOutput")
    if debug:
        henc_d = nc.dram_tensor("henc_dbg", [HID, LSHARD], f32, kind="ExternalOutput")
        tab_d = nc.dram_tensor("tab_dbg", [128, HID], f32, kind="ExternalOutput")
        agg_d = nc.dram_tensor("agg_dbg", [HID, LSHARD], f32, kind="ExternalOutput")
        conv_d = nc.dram_tensor("conv_dbg", [HID, LSHARD], f32, kind="ExternalOutput")
        h0_d = nc.dram_tensor("h0_dbg", [HID, LSHARD], f32, kind="ExternalOutput")

    rg = [list(range(NC))]

    def col_chunks(width=512):
        s = 0
        while s < LSHARD:
            w = min(width, LSHARD - s)
            yield s, w
            s += w

    with tile.TileContext(nc) as tc:
        with tc.tile_pool(name="persist", bufs=1) as pp, \
             tc.tile_pool(name="work", bufs=4) as wp, \
             tc.tile_pool(name="psum", bufs=2, space="PSUM") as psp, \
             tc.tile_pool(name="dram", bufs=1, space="DRAM") as dp:

            # ---- persistent SBUF ----
            hT = pp.tile([128, LSHARD], f32, name="hT")
            hpTb = pp.tile([128, LSHARD], bf16, name="hpTb")
            convT = pp.tile([128, LSHARD], f32, name="convT")
            dinv = pp.tile([128, LSHARD], f32, name="dinv")
            idx_sb = pp.tile([128, nchunks], i32, name="idx_sb")
            identb = pp.tile([128, 128], bf16, name="identb")
            wenc = pp.tile([F_IN, HID], f32, name="wenc")
            benc = pp.tile([HID, 1], f32, name="benc")
            wc = [pp.tile([HID, HID], bf16, name=f"wc{i}") for i in range(3)]
            bng = pp.tile([HID, 3], f32, name="bng")
            bnb = pp.tile([HID, 3], f32, name="bnb")
            wc1 = pp.tile([HID, 64], bf16, name="wc1")
            bc1 = pp.tile([64, 1], f32, name="bc1")
            wc2 = pp.tile([64, N_CLS], bf16, name="wc2")
            bc2 = pp.tile([N_CLS, 1], f32, name="bc2")
            bnst = pp.tile([128, BLOCKS * 6], f32, name="bnst")

            nc.sync.dma_start(out=dinv[:], in_=dinv_d.ap())
            nc.sync.dma_start(out=idx_sb[:], in_=idx_d.ap())
            nc.sync.dma_start(out=identb[:], in_=ident_d.ap())
            nc.sync.dma_start(out=wenc[:], in_=wenc_d.ap())
            nc.sync.dma_start(out=benc[:], in_=benc_d.ap())
            for l in range(3):
                nc.sync.dma_start(out=wc[l][:], in_=wc_d.ap()[l])
                nc.sync.dma_start(out=bng[:, l:l + 1], in_=bng_d.ap()[l])
                nc.sync.dma_start(out=bnb[:, l:l + 1], in_=bnb_d.ap()[l])
            nc.sync.dma_start(out=wc1[:], in_=wc1_d.ap())
            nc.sync.dma_start(out=bc1[:], in_=bc1_d.ap())
            nc.sync.dma_start(out=wc2[:], in_=wc2_d.ap())
            nc.sync.dma_start(out=bc2[:], in_=bc2_d.ap())

            # ---- encoder: hT = relu(Wenc^T @ xT + b) ----
            xT = pp.tile([F_IN, LSHARD], f32, name="xT")
            nc.sync.dma_start(out=xT[:], in_=xT_d.ap())
            for s, w in col_chunks():
                pse = psp.tile([128, 512], f32, tag="mm", name="pse")
                nc.tensor.matmul(out=pse[:, :w], lhsT=wenc[:], rhs=xT[:, s:s + w],
                                 start=True, stop=True)
                nc.scalar.activation(hT[:, s:s + w], pse[:, :w],
                                     mybir.ActivationFunctionType.Relu,
                                     bias=benc[:, 0:1], scale=1.0)

            if debug:
                nc.sync.dma_start(out=henc_d.ap(), in_=hT[:])
            # ---- conv layers ----
            for l in range(3):
                # h' = hT * dinv -> bf16
                for s, w in col_chunks():
                    nc.vector.tensor_tensor(out=hpTb[:, s:s + w], in0=hT[:, s:s + w],
                                            in1=dinv[:, s:s + w],
                                            op=mybir.AluOpType.mult)
                # transpose blocks to node-major bounce, allgather
                bounce = dp.tile([LSHARD, HID], bf16, name=f"bounce{l}")
                table = dp.tile([TOT, HID], bf16, addr_space="Shared",
                                name=f"table{l}")
                for b in range(BLOCKS):
                    bs = b * 128
                    pst = psp.tile([128, 128], f32, tag="tr", name="pst")
                    nc.tensor.matmul(out=pst[:], lhsT=hpTb[:, bs:bs + 128],
                                     rhs=identb[:], start=True, stop=True)
                    trs = wp.tile([128, 128], bf16, tag="trs", name="trs")
                    nc.vector.tensor_copy(out=trs[:], in_=pst[:])
                    nc.sync.dma_start(out=bounce[bs:bs + 128, :], in_=trs[:])
                nc.gpsimd.collective_compute(
                    "AllGather", mybir.AluOpType.bypass, replica_groups=rg,
                    ins=[bounce.opt()], outs=[table.opt()])
                if debug and l == 0:
                    tdbg = wp.tile([128, HID], bf16, tag="tdbg", name="tdbg")
                    nc.sync.dma_start(out=tdbg[:], in_=table[:128, :])
                    tdbg2 = wp.tile([128, HID], f32, tag="tdbg2", name="tdbg2")
                    nc.vector.tensor_copy(out=tdbg2[:], in_=tdbg[:])
                    nc.sync.dma_start(out=tab_d.ap(), in_=tdbg2[:])

                # gather calls
                gtiles = []
                ncalls = (nchunks + CG - 1) // CG
                for call in range(ncalls):
                    c0 = call * CG
                    cw = min(CG, nchunks - c0)
                    g = wp.tile([128, CG * 128], bf16, tag="g", name=f"g{l}_{call}")
                    nc.gpsimd.indirect_dma_start(
                        out=g[:, :cw * 128], out_offset=None,
                        in_=table[:],
                        in_offset=bass.IndirectOffsetOnAxis(
                            ap=idx_sb[:, c0:c0 + cw], axis=0))
                    gtiles.append(g)

                # per-block aggregation + conv + stats
                for b in range(BLOCKS):
                    bs = b * 128
                    psa = psp.tile([128, 128], f32, tag="agg", bufs=4, name="psa")
                    lo, hi = int(chunk_base[b]), int(chunk_base[b + 1])
                    for c in range(lo, hi):
                        gt = gtiles[c // CG]
                        j = c % CG
                        nc.tensor.matmul(out=psa[:],
                                         lhsT=gt[:, j * 128:(j + 1) * 128],
                                         rhs=identb[:],
                                         start=(c == lo), stop=(c == hi - 1))
                    at = wp.tile([128, 128], bf16, tag="at", name="at")
                    nc.vector.tensor_tensor(out=at[:], in0=psa[:],
                                            in1=hpTb[:, bs:bs + 128],
                                            op=mybir.AluOpType.add)
                    if debug and l == 0:
                        atf = wp.tile([128, 128], f32, tag="atf", name="atf")
                        nc.vector.tensor_copy(out=atf[:], in_=at[:])
                        nc.sync.dma_start(out=agg_d.ap()[:, bs:bs + 128], in_=atf[:])
                    psc = psp.tile([128, 128], f32, tag="mm", name="psc")
                    nc.tensor.matmul(out=psc[:], lhsT=wc[l][:], rhs=at[:],
                                     start=True, stop=True)
                    nc.vector.tensor_tensor(out=convT[:, bs:bs + 128], in0=psc[:],
                                            in1=dinv[:, bs:bs + 128],
                                            op=mybir.AluOpType.mult)
                    nc.vector.bn_stats(out=bnst[:, b * 6:(b + 1) * 6],
                                       in_=convT[:, bs:bs + 128])

                # global BN stats
                bnagg = wp.tile([128, 2], f32, tag="st", name="bnagg")
                nc.vector.bn_aggr(out=bnagg[:], in_=bnst[:])
                ssum = wp.tile([128, 2], f32, tag="st", name="ssum")
                # ssum[:,0] = LSHARD*mean ; ssum[:,1] = LSHARD*(var+mean^2)
                m2 = wp.tile([128, 1], f32, tag="st1", name="m2")
                nc.vector.tensor_tensor(out=m2[:], in0=bnagg[:, 0:1],
                                        in1=bnagg[:, 0:1], op=mybir.AluOpType.mult)
                nc.vector.tensor_scalar_mul(ssum[:, 0:1], bnagg[:, 0:1],
                                            float(LSHARD))
                q = wp.tile([128, 1], f32, tag="st1", name="q")
                nc.vector.tensor_tensor(out=q[:], in0=bnagg[:, 1:2], in1=m2[:],
                                        op=mybir.AluOpType.add)
                nc.vector.tensor_scalar_mul(ssum[:, 1:2], q[:], float(LSHARD))
                stat_src = dp.tile([128, 2], f32, name=f"stat_src{l}")
                stat_dst = dp.tile([128, 2], f32, addr_space="Shared",
                                   name=f"stat_dst{l}")
                nc.sync.dma_start(out=stat_src[:], in_=ssum[:])
                nc.gpsimd.collective_compute(
                    "AllReduce", mybir.AluOpType.add, replica_groups=rg,
                    ins=[stat_src.opt()], outs=[stat_dst.opt()])
                gstat = wp.tile([128, 2], f32, tag="st", name="gstat")
                nc.sync.dma_start(out=gstat[:], in_=stat_dst[:])
                mu = wp.tile([128, 1], f32, tag="st1", name="mu")
                nc.vector.tensor_scalar_mul(mu[:], gstat[:, 0:1], 1.0 / N_REAL)
                var = wp.tile([128, 1], f32, tag="st1", name="var")
                nc.vector.tensor_scalar_mul(var[:], gstat[:, 1:2], 1.0 / N_REAL)
                mu2 = wp.tile([128, 1], f32, tag="st1", name="mu2")
                nc.vector.tensor_tensor(out=mu2[:], in0=mu[:], in1=mu[:],
                                        op=mybir.AluOpType.mult)
                nc.vector.tensor_tensor(out=var[:], in0=var[:], in1=mu2[:],
                                        op=mybir.AluOpType.subtract)
                nc.vector.tensor_scalar_add(var[:], var[:], EPS)
                rinv = wp.tile([128, 1], f32, tag="st1", name="rinv")
                nc.vector.reciprocal(rinv[:], var[:])
                rs = wp.tile([128, 1], f32, tag="st1", name="rs")
                nc.scalar.sqrt(rs[:], rinv[:])
                gp = wp.tile([128, 1], f32, tag="st1", name="gp")
                nc.vector.tensor_tensor(out=gp[:], in0=bng[:, l:l + 1], in1=rs[:],
                                        op=mybir.AluOpType.mult)
                mgp = wp.tile([128, 1], f32, tag="st1", name="mgp")
                nc.vector.tensor_tensor(out=mgp[:], in0=mu[:], in1=gp[:],
                                        op=mybir.AluOpType.mult)
                bp = wp.tile([128, 1], f32, tag="st1", name="bp")
                nc.vector.tensor_tensor(out=bp[:], in0=bnb[:, l:l + 1], in1=mgp[:],
                                        op=mybir.AluOpType.subtract)

                if debug and l == 0:
                    nc.sync.dma_start(out=conv_d.ap(), in_=convT[:])
                # bn apply + relu (+ residual)
                for s, w in col_chunks():
                    if l == 0:
                        nc.scalar.activation(hT[:, s:s + w], convT[:, s:s + w],
                                             mybir.ActivationFunctionType.Relu,
                                             bias=bp[:, 0:1], scale=gp[:, 0:1])
                    else:
                        hnew = wp.tile([128, 512], f32, tag="hnew", name="hnew")
                        nc.scalar.activation(hnew[:, :w], convT[:, s:s + w],
                                             mybir.ActivationFunctionType.Relu,
                                             bias=bp[:, 0:1], scale=gp[:, 0:1])
                        nc.vector.tensor_tensor(out=hT[:, s:s + w],
                                                in0=hT[:, s:s + w],
                                                in1=hnew[:, :w],
                                                op=mybir.AluOpType.add)

            if debug:
                nc.sync.dma_start(out=h0_d.ap(), in_=hT[:])
            # ---- classifier ----
            h4b = pp.tile([64, LSHARD], bf16, name="h4b")
            for s, w in col_chunks():
                nc.vector.tensor_copy(out=hpTb[:, s:s + w], in_=hT[:, s:s + w])
                ps1 = psp.tile([64, 512], f32, tag="mm", name="ps1")
                nc.tensor.matmul(out=ps1[:, :w], lhsT=wc1[:], rhs=hpTb[:, s:s + w],
                                 start=True, stop=True)
                nc.scalar.activation(h4b[:, s:s + w], ps1[:, :w],
                                     mybir.ActivationFunctionType.Relu,
                                     bias=bc1[:, 0:1], scale=1.0)
            for s, w in col_chunks():
                ps2 = psp.tile([N_CLS, 512], f32, tag="mm", name="ps2")
                nc.tensor.matmul(out=ps2[:, :w], lhsT=wc2[:], rhs=h4b[:, s:s + w],
                                 start=True, stop=True)
                ot = wp.tile([N_CLS, 512], f32, tag="ot", name="ot")
                nc.vector.tensor_scalar(out=ot[:, :w], in0=ps2[:, :w],
                                        scalar1=bc2[:, 0:1], scalar2=None,
                                        op0=mybir.AluOpType.add)
                nc.sync.dma_start(out=out_d.ap()[:, s:s + w], in_=ot[:, :w])

    nc.compile()
    return nc


def make_host_inputs(inputs, P):
    """Per-core input dicts from full inputs + preprocessing P."""
    import ml_dtypes
    nbf = ml_dtypes.bfloat16
    x = np.asarray(inputs["x"], np.float32)
    core_nodes = P["core_nodes"]
    dinv = P["dinv"]
    ident = np.eye(128, dtype=nbf)
    wconv = np.stack([np.asarray(inputs[f"W_conv{i}"], np.float32)
                      for i in range(3)]).astype(nbf)
    bng = np.stack([np.asarray(inputs[f"bn_g{i}"], np.float32)
                    for i in range(3)])[:, :, None]
    bnb = np.stack([np.asarray(inputs[f"bn_b{i}"], np.float32)
                    for i in range(3)])[:, :, None]
    maps = []
    for c in range(NC):
        xT = np.zeros((F_IN, LSHARD), np.float32)
        xT[:, :PER_CORE] = x[core_nodes[c]].T
        dl = np.zeros((LSHARD,), np.float32)
        dl[:PER_CORE] = dinv[core_nodes[c]]
        maps.append({
            "xT": xT,
            "idx": P["idx"][c],
            "dinvbc": np.broadcast_to(dl, (128, LSHARD)).copy(),
            "W_enc": np.asarray(inputs["W_enc"], np.float32),
            "b_enc": np.asarray(inputs["b_enc"], np.float32)[:, None],
            "W_conv": wconv,
            "bn_g": bng.astype(np.float32),
            "bn_b": bnb.astype(np.float32),
            "W_cls1": np.asarray(inputs["W_cls1"], np.float32).astype(nbf),
            "b_cls1": np.asarray(inputs["b_cls1"], np.float32)[:, None],
            "W_cls2": np.asarray(inputs["W_cls2"], np.float32).astype(nbf),
            "b_cls2": np.asarray(inputs["b_cls2"], np.float32)[:, None],
            "ident": ident,
        })
    return maps


def assemble_output(results, P):
    out = np.zeros((N_REAL, N_CLS), np.float32)
    for c in range(NC):
        out[P["core_nodes"][c]] = results[c]["outT"][:, :PER_CORE].T
    return out


# ---------------- host preprocessing ----------------
def preprocess(edge_index):
    src = np.asarray(edge_index[0], dtype=np.int64)
    dst = np.asarray(edge_index[1], dtype=np.int64)
    indeg = np.bincount(dst, minlength=N).astype(np.int64)
    deg = (indeg + 1).astype(np.float32)
    dinv = (1.0 / np.sqrt(deg)).astype(np.float32)

    # snake-deal nodes sorted by indeg desc -> 8 cores x 6250, edge-balanced
    order = np.argsort(-indeg, kind="stable")
    rounds = PER_CORE // 2  # deal in rounds of 16 (8 fwd + 8 back)
    assert rounds * 2 * NC == N
    core_nodes = [[] for _ in range(NC)]
    pos = 0
    for r in range(rounds):
        for c in range(NC):
            core_nodes[c].append(order[pos]); pos += 1
        for c in range(NC - 1, -1, -1):
            core_nodes[c].append(order[pos]); pos += 1
    core_nodes = np.array(core_nodes)  # [NC, 6250] global ids
    # within each core sort by indeg desc (for tight block max-degree)
    for c in range(NC):
        o = np.argsort(-indeg[core_nodes[c]], kind="stable")
        core_nodes[c] = core_nodes[c][o]

    # global id -> table row (core*LSHARD + local)
    table_row = np.full(N, -1, dtype=np.int64)
    for c in range(NC):
        table_row[core_nodes[c]] = c * LSHARD + np.arange(PER_CORE)

    # block max degrees, shared across cores
    ind_local = indeg[core_nodes]  # [NC, 6250]
    ind_pad = np.zeros((NC, LSHARD), dtype=np.int64)
    ind_pad[:, :PER_CORE] = ind_local
    Db = ind_pad.reshape(NC, BLOCKS, 128).max(axis=2).max(axis=0)  # [BLOCKS]
    Db = np.maximum(Db, 1).astype(np.int64)
    nchunks = int(Db.sum())

    # slot tables per core: idx[p, chunk] = table row of the k-th in-edge of
    # local dst (b*128+p), or the core's zero row when k >= indeg
    zero_row = np.array([c * LSHARD + PER_CORE for c in range(NC)], dtype=np.int64)
    idx = np.zeros((NC, 128, nchunks), dtype=np.int32)
    for c in range(NC):
        idx[c, :, :] = zero_row[c]
    # bucket edges by dst
    e_order = np.argsort(dst, kind="stable")
    s_sorted = src[e_order]
    d_sorted = dst[e_order]
    starts = np.searchsorted(d_sorted, np.arange(N))
    ends = np.searchsorted(d_sorted, np.arange(N) + 1)
    chunk_base = np.concatenate([[0], np.cumsum(Db)])  # chunk index base per block
    for c in range(NC):
        for j in range(PER_CORE):
            g = core_nodes[c, j]
            b, p = j // 128, j % 128
            lo, hi = starts[g], ends[g]
            k = hi - lo
            if k:
                idx[c, p, chunk_base[b]:chunk_base[b] + k] = table_row[s_sorted[lo:hi]]

    waste = nchunks * 128 / (E / NC)
    return dict(src=src, dst=dst, dinv=dinv, core_nodes=core_nodes,
                table_row=table_row, Db=Db, nchunks=nchunks, idx=idx,
                chunk_base=chunk_base, zero_row=zero_row, waste=waste)




# ---------------- SPMD runner ----------------
class SpmdRunner:
    def __init__(self, nc, n_cores: int, donate: bool = True):
        install_neuronx_cc_hook()
        self.nc = nc
        self.n_cores = n_cores
        partition_name = nc.partition_id_tensor.name if nc.partition_id_tensor else None

        in_names: list[str] = []
        out_names: list[str] = []
        out_avals = []
        zero_outs: list[np.ndarray] = []
        for alloc in nc.m.functions[0].allocations:
            if not isinstance(alloc, mybir.MemoryLocationSet):
                continue
            name = alloc.memorylocations[0].name
            if alloc.kind == "ExternalInput":
                if name != partition_name:
                    in_names.append(name)
            elif alloc.kind == "ExternalOutput":
                shape = tuple(alloc.tensor_shape)
                dtype = mybir.dt.np(alloc.dtype)
                out_names.append(name)
                out_avals.append(jax.core.ShapedArray(shape, dtype))
                zero_outs.append(np.zeros(shape, dtype))
        self.in_names = in_names
        self.out_names = out_names
        self.out_avals = out_avals
        self.zero_outs = zero_outs
        n_params = len(in_names)
        n_outs = len(out_avals)
        all_names = list(in_names) + list(out_names)
        if partition_name is not None:
            all_names.append(partition_name)

        def _body(*args):
            operands = list(args)
            if partition_name is not None:
                operands.append(partition_id_tensor())
            outs = _bass_exec_p.bind(
                *operands,
                out_avals=tuple(out_avals),
                in_names=tuple(all_names),
                out_names=tuple(out_names),
                lowering_input_output_aliases=(),
                sim_require_finite=True,
                sim_require_nnan=True,
                nc=nc,
            )
            return tuple(outs)

        devices = jax.devices()[:n_cores]
        assert len(devices) == n_cores
        self.mesh = Mesh(np.asarray(devices), ("core",))
        in_specs = (PartitionSpec("core"),) * (n_params + n_outs)
        out_specs = (PartitionSpec("core"),) * n_outs
        donate_argnums = tuple(range(n_params, n_params + n_outs)) if donate else ()
        self.fn = jax.jit(
            shard_map(_body, mesh=self.mesh, in_specs=in_specs,
                      out_specs=out_specs, check_rep=False),
            donate_argnums=donate_argnums,
            keep_unused=True,
        )

    def concat_inputs(self, in_maps):
        n = self.n_cores
        return [
            np.concatenate([np.asarray(in_maps[c][name]) for c in range(n)], axis=0)
            for name in self.in_names
        ]

    def concat_zeros(self):
        return [np.zeros((self.n_cores * z.shape[0], *z.shape[1:]), z.dtype)
                for z in self.zero_outs]

    def run(self, in_maps):
        """Execute once; returns list (per core) of dicts name->np.ndarray."""
        concat_in = self.concat_inputs(in_maps)
        out_arrs = self.fn(*concat_in, *self.concat_zeros())
        res = []
        for c in range(self.n_cores):
            res.append({
                name: np.asarray(out_arrs[i]).reshape(
                    self.n_cores, *self.out_avals[i].shape)[c]
                for i, name in enumerate(self.out_names)
            })
        return res

    def timeit(self, in_maps, iters: int = 20, warmup: int = 3):
        """Wall-clock per-iteration time in ns, device-resident inputs."""
        concat_in = [jax.device_put(x) for x in self.concat_inputs(in_maps)]
        # pre-stage zero output buffers for every iteration (donated)
        all_zero_sets = [
            [jax.device_put(z) for z in self.concat_zeros()]
            for _ in range(iters + warmup)
        ]
        for x in concat_in:
            x.block_until_ready()
        for zs in all_zero_sets:
            for z in zs:
                z.block_until_ready()
        outs = None
        for i in range(warmup):
            outs = self.fn(*concat_in, *all_zero_sets[i])
        if outs:
            jax.block_until_ready(outs)
        t0 = time.perf_counter()
        for i in range(iters):
            outs = self.fn(*concat_in, *all_zero_sets[warmup + i])
        jax.block_until_ready(outs)
        t1 = time.perf_counter()
        return (t1 - t0) / iters * 1e9


_CACHE = {}


def kernel(**inputs):
    inputs = {k: np.asarray(v) for k, v in inputs.items()}
    P = preprocess(inputs["edge_index"])
    key = (int(P["nchunks"]), tuple(int(d) for d in P["Db"]))
    if key not in _CACHE:
        nc = build(P["Db"], P["nchunks"])
        _CACHE[key] = SpmdRunner(nc, NC)
    r = _CACHE[key]
    in_maps = make_host_inputs(inputs, P)
    res = r.run(in_maps)
    return assemble_output(res, P)



# revision 17
# speedup vs baseline: 1.1357x; 1.1357x over previous
"""Self-contained 8-core Trainium2 Bass kernel for the BaseGNN problem.

kernel(**inputs) -> np.ndarray [50000, 72] float32.

v2 strategy: degree-sorted node sharding across 8 NeuronCores. Per conv
layer, h' = dinv*h is allgathered as a bf16 node-major table in DRAM
(two halves so row ids fit int16), edge messages are fetched with a few
large batched dma_gather calls (instead of ~1000 small indirect DMAs),
aggregated per 128-dst-node block on the TensorEngine into PSUM,
normalized (BN stats via AllReduce) and activated; encoder/classifier
matmuls are fused in, epilogues batched 512 columns at a time.
"""
import numpy as np
import ml_dtypes

import jax
from jax.sharding import Mesh, PartitionSpec
from jax.experimental.shard_map import shard_map

import concourse.bacc as bacc
import concourse.tile as tile
import concourse.mybir as mybir
from concourse import bass
from concourse.bass2jax import _bass_exec_p, install_neuronx_cc_hook, partition_id_tensor

N = 50000
E = 1000000
bf16_np = ml_dtypes.bfloat16

F_IN = 16
HID = 128
N_CLS = 72
EPS = 1e-5
NC = 8
PER_CORE = 6250
BLOCKS = 49
LSHARD = BLOCKS * 128
N_REAL = 50000

# --- v2 half-table layout (int16-safe row ids for dma_gather) ---
A_BLOCKS = 25
B_BLOCKS = BLOCKS - A_BLOCKS
HALF_A_ROWS = A_BLOCKS * 128       # 3200 local rows in half A
HALF_B_ROWS = B_BLOCKS * 128       # 3072 local rows in half B
NODES_A = HALF_A_ROWS - 1          # 3199 real nodes; local row 3199 is A's zero row
NODES_B = PER_CORE - NODES_A       # 3051 real nodes; local rows 6251.. are B zeros
TOT_A = NC * HALF_A_ROWS           # 25600 (< 32768)
TOT_B = NC * HALF_B_ROWS           # 24576 (< 32768)
ZROW_A = NODES_A                   # core 0's A pad row
ZROW_B = NODES_B                   # core 0's B pad row, rebased
CAP = 64                           # gather buffer capacity (chunks, per half)
GROUP = 4                          # blocks per epilogue group (512 cols)

f32 = mybir.dt.float32
bf16 = mybir.dt.bfloat16
i16 = mybir.dt.int16


# ---------------- host preprocessing ----------------
def preprocess(edge_index):
    src = np.asarray(edge_index[0], dtype=np.int64)
    dst = np.asarray(edge_index[1], dtype=np.int64)
    indeg = np.bincount(dst, minlength=N).astype(np.int64)
    deg = (indeg + 1).astype(np.float32)
    dinv = (1.0 / np.sqrt(deg)).astype(np.float32)

    # half membership: global top NC*NODES_A nodes by indeg -> A
    order = np.argsort(-indeg, kind="stable")
    isA = np.zeros(N, dtype=bool)
    isA[order[:NC * NODES_A]] = True
    # per-dst counts of in-edges by source half
    dA = np.bincount(dst[isA[src]], minlength=N).astype(np.int64)
    dB = indeg - dA

    # snake-deal nodes in (indeg desc, A-deg desc) order -> 8 cores x 6250;
    # position j of every core holds global ranks 8j..8j+7, so per-core
    # degree profiles align rank-by-rank (tight shared block maxima) and
    # the first NODES_A positions of each core are exactly the A half.
    # boustrophedon over (indeg, dA): alternate dA direction per degree run so
    # dA stays continuous across run boundaries (tight per-block maxima for
    # both halves even when a block straddles a degree boundary)
    dA_snake = np.where(indeg % 2 == 0, -dA, dA)
    ord2 = np.lexsort((dA_snake, (~isA).astype(np.int64), -indeg))
    rounds = PER_CORE // 2
    assert rounds * 2 * NC == N
    core_nodes = np.empty((NC, PER_CORE), dtype=np.int64)
    fwd = ord2.reshape(rounds * 2, NC)
    core_nodes[:, 0::2] = fwd[0::2].T
    core_nodes[:, 1::2] = fwd[1::2][:, ::-1].T
    assert isA[core_nodes[:, :NODES_A]].all()
    assert not isA[core_nodes[:, NODES_A:]].any()

    # local rows: positions 0..NODES_A-1 -> rows 0..3198; rest -> 3200..6250
    local_rows = np.concatenate([np.arange(NODES_A),
                                 HALF_A_ROWS + np.arange(NODES_B)])
    core_of = np.empty(N, dtype=np.int64)
    lrow_of = np.empty(N, dtype=np.int64)
    for c in range(NC):
        core_of[core_nodes[c]] = c
        lrow_of[core_nodes[c]] = local_rows

    # int16 table rows per half
    rowh = np.empty(N, dtype=np.int64)
    a_mask_node = lrow_of < HALF_A_ROWS
    rowh[a_mask_node] = core_of[a_mask_node] * HALF_A_ROWS + lrow_of[a_mask_node]
    bm = ~a_mask_node
    rowh[bm] = core_of[bm] * HALF_B_ROWS + (lrow_of[bm] - HALF_A_ROWS)

    # per-(core, block) chunk counts shared across cores
    dA_pad = np.zeros((NC, LSHARD), dtype=np.int64)
    dB_pad = np.zeros((NC, LSHARD), dtype=np.int64)
    for c in range(NC):
        dA_pad[c, local_rows] = dA[core_nodes[c]]
        dB_pad[c, local_rows] = dB[core_nodes[c]]
    DbA = dA_pad.reshape(NC, BLOCKS, 128).max(axis=(0, 2))
    DbB = dB_pad.reshape(NC, BLOCKS, 128).max(axis=(0, 2))
    both_zero = (DbA + DbB) == 0
    DbB[both_zero] = 1
    cbA = np.concatenate([[0], np.cumsum(DbA)]).astype(np.int64)
    cbB = np.concatenate([[0], np.cumsum(DbB)]).astype(np.int64)
    nchA = int(cbA[-1])
    nchB = int(cbB[-1])

    # vectorized slot-stream construction
    blk_of = lrow_of // 128
    p_of = lrow_of % 128

    def build_stream(sel_mask, cb, nch, zrow, half_is_a):
        e_sel = np.nonzero(sel_mask)[0]
        d_sel = dst[e_sel]
        o = np.argsort(d_sel, kind="stable")
        e_sel = e_sel[o]
        d_sel = d_sel[o]
        s_sel = src[e_sel]
        # rank of each edge within its dst
        starts = np.searchsorted(d_sel, np.arange(N))
        rank = np.arange(len(e_sel)) - starts[d_sel]
        slot = (cb[blk_of[d_sel]] + rank) * 128 + p_of[d_sel]
        stream = np.full((NC, nch * 128), zrow, dtype=np.int16)
        val = rowh[s_sel]
        if half_is_a:
            assert (val < TOT_A).all()
        else:
            assert (val < TOT_B).all()
        stream[core_of[d_sel], slot] = val.astype(np.int16)
        return stream

    streamA = build_stream(isA[src], cbA, nchA, ZROW_A, True)
    streamB = build_stream(~isA[src], cbB, nchB, ZROW_B, False)

    def wrap16(stream):
        # [NC, nch*128] -> [NC, 128, nch*8]; value for position j at
        # (j % 16, j // 16), replicated across the 8 groups of 16 partitions
        ncol = stream.shape[1] // 16
        w = stream.reshape(NC, ncol, 16).transpose(0, 2, 1)  # [NC, 16, ncol]
        return np.tile(w, (1, 8, 1)).copy()

    idxA = wrap16(streamA)
    idxB = wrap16(streamB)

    # slabs: greedy ranges of blocks with per-half chunk counts <= CAP
    slabs = []
    b0 = 0
    while b0 < BLOCKS:
        b1 = b0 + 1
        while (b1 < BLOCKS
               and cbA[b1 + 1] - cbA[b0] <= CAP
               and cbB[b1 + 1] - cbB[b0] <= CAP):
            b1 += 1
        assert cbA[b1] - cbA[b0] <= CAP and cbB[b1] - cbB[b0] <= CAP
        slabs.append((b0, b1, int(cbA[b0]), int(cbA[b1]),
                      int(cbB[b0]), int(cbB[b1])))
        b0 = b1

    waste = (nchA + nchB) * 128 / (E / NC)
    return dict(dinv=dinv, core_nodes=core_nodes, local_rows=local_rows,
                DbA=DbA, DbB=DbB, cbA=cbA, cbB=cbB, nchA=nchA, nchB=nchB,
                idxA=idxA, idxB=idxB, slabs=slabs, waste=waste)


# ---------------- kernel build ----------------
def build(P):
    cbA, cbB = P["cbA"], P["cbB"]
    nchA, nchB = P["nchA"], P["nchB"]
    slabs = P["slabs"]
    SA = nchA * 8
    SB = nchB * 8
    nslab = len(slabs)

    nc = bacc.Bacc("TRN2", target_bir_lowering=False, debug=False,
                   enable_asserts=True, num_devices=NC)

    # ---- inputs ----
    xT_d = nc.dram_tensor("xT", [F_IN, LSHARD], f32, kind="ExternalInput")
    idxA_d = nc.dram_tensor("idxA", [128, SA], i16, kind="ExternalInput")
    idxB_d = nc.dram_tensor("idxB", [128, SB], i16, kind="ExternalInput")
    dinv_d = nc.dram_tensor("dinvbc", [128, LSHARD], f32, kind="ExternalInput")
    wenc_d = nc.dram_tensor("W_enc", [F_IN, HID], f32, kind="ExternalInput")
    benc_d = nc.dram_tensor("b_enc", [HID, 1], f32, kind="ExternalInput")
    wc_d = nc.dram_tensor("W_conv", [3, HID, HID], bf16, kind="ExternalInput")
    bng_d = nc.dram_tensor("bn_g", [3, HID, 1], f32, kind="ExternalInput")
    bnb_d = nc.dram_tensor("bn_b", [3, HID, 1], f32, kind="ExternalInput")
    wc1_d = nc.dram_tensor("W_cls1", [HID, 64], bf16, kind="ExternalInput")
    bc1_d = nc.dram_tensor("b_cls1", [64, 1], f32, kind="ExternalInput")
    wc2_d = nc.dram_tensor("W_cls2", [64, N_CLS], bf16, kind="ExternalInput")
    bc2_d = nc.dram_tensor("b_cls2", [N_CLS, 1], f32, kind="ExternalInput")
    ident_d = nc.dram_tensor("ident", [128, 128], bf16, kind="ExternalInput")
    out_d = nc.dram_tensor("outT", [N_CLS, LSHARD], f32, kind="ExternalOutput")

    rg = [list(range(NC))]

    # column groups for 512-wide epilogues: 12 x 512 + 1 x 128
    groups = []
    s = 0
    while s < LSHARD:
        w = min(GROUP * 128, LSHARD - s)
        groups.append((s, w))
        s += w

    with tile.TileContext(nc) as tc:
        with tc.tile_pool(name="persist", bufs=1) as pp, \
             tc.tile_pool(name="work", bufs=4) as wp, \
             tc.tile_pool(name="psum", bufs=2, space="PSUM") as psp, \
             tc.tile_pool(name="dram", bufs=1, space="DRAM") as dp:

            # ---- persistent SBUF ----
            hT = pp.tile([128, LSHARD], f32, name="hT")
            hpTb = pp.tile([128, LSHARD], bf16, name="hpTb")
            convTb = pp.tile([128, LSHARD], bf16, name="convTb")
            dinv = pp.tile([128, LSHARD], f32, name="dinv")
            idxA_sb = pp.tile([128, SA], i16, name="idxA_sb")
            idxB_sb = pp.tile([128, SB], i16, name="idxB_sb")
            identb = pp.tile([128, 128], bf16, name="identb")
            wenc = pp.tile([F_IN, HID], f32, name="wenc")
            benc = pp.tile([HID, 1], f32, name="benc")
            wc = [pp.tile([HID, HID], bf16, name=f"wc{i}") for i in range(3)]
            bng = pp.tile([HID, 3], f32, name="bng")
            bnb = pp.tile([HID, 3], f32, name="bnb")
            wc1 = pp.tile([HID, 64], bf16, name="wc1")
            bc1 = pp.tile([64, 1], f32, name="bc1")
            wc2 = pp.tile([64, N_CLS], bf16, name="wc2")
            bc2 = pp.tile([N_CLS, 1], f32, name="bc2")
            bnst = pp.tile([128, len(groups) * 6], f32, name="bnst")

            nc.sync.dma_start(out=dinv[:], in_=dinv_d.ap())
            nc.sync.dma_start(out=idxA_sb[:], in_=idxA_d.ap())
            nc.sync.dma_start(out=idxB_sb[:], in_=idxB_d.ap())
            nc.sync.dma_start(out=identb[:], in_=ident_d.ap())
            nc.sync.dma_start(out=wenc[:], in_=wenc_d.ap())
            nc.sync.dma_start(out=benc[:], in_=benc_d.ap())
            for l in range(3):
                nc.sync.dma_start(out=wc[l][:], in_=wc_d.ap()[l])
                nc.sync.dma_start(out=bng[:, l:l + 1], in_=bng_d.ap()[l])
                nc.sync.dma_start(out=bnb[:, l:l + 1], in_=bnb_d.ap()[l])
            nc.sync.dma_start(out=wc1[:], in_=wc1_d.ap())
            nc.sync.dma_start(out=bc1[:], in_=bc1_d.ap())
            nc.sync.dma_start(out=wc2[:], in_=wc2_d.ap())
            nc.sync.dma_start(out=bc2[:], in_=bc2_d.ap())

            # ---- encoder: hT = relu(Wenc^T @ xT + b), xT streamed per chunk ----
            for gs, w in groups:
                xch = wp.tile([F_IN, 512], f32, tag="xch", name="xch")
                nc.sync.dma_start(out=xch[:, :w], in_=xT_d.ap()[:, gs:gs + w])
                pse = psp.tile([128, 512], f32, tag="mm", name="pse")
                nc.tensor.matmul(out=pse[:, :w], lhsT=wenc[:],
                                 rhs=xch[:, :w], start=True, stop=True)
                nc.scalar.activation(hT[:, gs:gs + w], pse[:, :w],
                                     mybir.ActivationFunctionType.Relu,
                                     bias=benc[:, 0:1], scale=1.0)

            # ---- conv layers ----
            for l in range(3):
                bounceA = dp.tile([HALF_A_ROWS, HID], bf16, name=f"bounceA{l}")
                bounceB = dp.tile([HALF_B_ROWS, HID], bf16, name=f"bounceB{l}")
                tableA = dp.tile([TOT_A, HID], bf16, addr_space="Shared",
                                 name=f"tableA{l}")
                tableB = dp.tile([TOT_B, HID], bf16, addr_space="Shared",
                                 name=f"tableB{l}")

                # h' = hT * dinv -> bf16; transpose to node-major; bounce
                for gi, (gs, w) in enumerate(groups):
                    nc.vector.tensor_tensor(out=hpTb[:, gs:gs + w],
                                            in0=hT[:, gs:gs + w],
                                            in1=dinv[:, gs:gs + w],
                                            op=mybir.AluOpType.mult)
                    pst = psp.tile([128, 512], f32, tag="tr", name="pst")
                    nblk = w // 128
                    for j in range(nblk):
                        nc.tensor.matmul(out=pst[:, j * 128:(j + 1) * 128],
                                         lhsT=hpTb[:, gs + j * 128:gs + (j + 1) * 128],
                                         rhs=identb[:], start=True, stop=True)
                    trs = wp.tile([128, 512], bf16, tag="trs", name="trs")
                    nc.scalar.copy(out=trs[:, :w], in_=pst[:, :w])
                    for j in range(nblk):
                        b = gi * GROUP + j
                        bs = b * 128
                        if b < A_BLOCKS:
                            nc.sync.dma_start(
                                out=bounceA[bs:bs + 128, :],
                                in_=trs[:, j * 128:(j + 1) * 128])
                        else:
                            bs2 = (b - A_BLOCKS) * 128
                            nc.sync.dma_start(
                                out=bounceB[bs2:bs2 + 128, :],
                                in_=trs[:, j * 128:(j + 1) * 128])

                nc.gpsimd.collective_compute(
                    "AllGather", mybir.AluOpType.bypass, replica_groups=rg,
                    ins=[bounceA.opt()], outs=[tableA.opt()])
                nc.gpsimd.collective_compute(
                    "AllGather", mybir.AluOpType.bypass, replica_groups=rg,
                    ins=[bounceB.opt()], outs=[tableB.opt()])

                # batched gathers per slab
                gA_tiles = [None] * nslab
                gB_tiles = [None] * nslab
                for si, (b0, b1, a0, a1, bb0, bb1) in enumerate(slabs):
                    na = a1 - a0
                    nb = bb1 - bb0
                    if na > 0:
                        gA = wp.tile([128, CAP, 128], bf16, tag="gA",
                                     bufs=2, name=f"gA{l}_{si}")
                        nc.gpsimd.dma_gather(
                            out_ap=gA[:, :na, :], in_ap=tableA[:],
                            idxs_ap=idxA_sb[:, a0 * 8:a1 * 8],
                            num_idxs=na * 128, num_idxs_reg=na * 128,
                            elem_size=HID, single_packet=False)
                        gA_tiles[si] = gA
                    if nb > 0:
                        gB = wp.tile([128, CAP, 128], bf16, tag="gB",
                                     bufs=2, name=f"gB{l}_{si}")
                        nc.gpsimd.dma_gather(
                            out_ap=gB[:, :nb, :], in_ap=tableB[:],
                            idxs_ap=idxB_sb[:, bb0 * 8:bb1 * 8],
                            num_idxs=nb * 128, num_idxs_reg=nb * 128,
                            elem_size=HID, single_packet=False)
                        gB_tiles[si] = gB

                blk_slab = np.empty(BLOCKS, dtype=np.int64)
                for si, (b0, b1, *_rest) in enumerate(slabs):
                    blk_slab[b0:b1] = si

                # per-group aggregation + conv + stats
                for gi, (gs, w) in enumerate(groups):
                    nblk = w // 128
                    psa = psp.tile([128, 512], f32, tag="agg", name="psa")
                    for j in range(nblk):
                        b = gi * GROUP + j
                        si = int(blk_slab[b])
                        _, _, a0, _, bb0, _ = slabs[si]
                        seq = []
                        for c in range(int(cbA[b]), int(cbA[b + 1])):
                            seq.append((gA_tiles[si], c - a0))
                        for c in range(int(cbB[b]), int(cbB[b + 1])):
                            seq.append((gB_tiles[si], c - bb0))
                        assert seq, f"block {b} has no chunks"
                        for k, (gt, off) in enumerate(seq):
                            nc.tensor.matmul(
                                out=psa[:, j * 128:(j + 1) * 128],
                                lhsT=gt[:, off, :], rhs=identb[:],
                                start=(k == 0), stop=(k == len(seq) - 1))
                    at = wp.tile([128, 512], bf16, tag="at", name="at")
                    nc.vector.tensor_tensor(out=at[:, :w], in0=psa[:, :w],
                                            in1=hpTb[:, gs:gs + w],
                                            op=mybir.AluOpType.add)
                    psc = psp.tile([128, 512], f32, tag="mm", name="psc")
                    nc.tensor.matmul(out=psc[:, :w], lhsT=wc[l][:],
                                     rhs=at[:, :w], start=True, stop=True)
                    nc.vector.tensor_tensor(out=convTb[:, gs:gs + w],
                                            in0=psc[:, :w],
                                            in1=dinv[:, gs:gs + w],
                                            op=mybir.AluOpType.mult)
                    nc.vector.bn_stats(out=bnst[:, gi * 6:(gi + 1) * 6],
                                       in_=convTb[:, gs:gs + w])

                # global BN stats
                bnagg = wp.tile([128, 2], f32, tag="st", name="bnagg")
                nc.vector.bn_aggr(out=bnagg[:], in_=bnst[:])
                ssum = wp.tile([128, 2], f32, tag="st", name="ssum")
                m2 = wp.tile([128, 1], f32, tag="st1", name="m2")
                nc.vector.tensor_tensor(out=m2[:], in0=bnagg[:, 0:1],
                                        in1=bnagg[:, 0:1], op=mybir.AluOpType.mult)
                nc.vector.tensor_scalar_mul(ssum[:, 0:1], bnagg[:, 0:1],
                                            float(LSHARD))
                q = wp.tile([128, 1], f32, tag="st1", name="q")
                nc.vector.tensor_tensor(out=q[:], in0=bnagg[:, 1:2], in1=m2[:],
                                        op=mybir.AluOpType.add)
                nc.vector.tensor_scalar_mul(ssum[:, 1:2], q[:], float(LSHARD))
                stat_src = dp.tile([128, 2], f32, name=f"stat_src{l}")
                stat_dst = dp.tile([128, 2], f32, addr_space="Shared",
                                   name=f"stat_dst{l}")
                nc.sync.dma_start(out=stat_src[:], in_=ssum[:])
                nc.gpsimd.collective_compute(
                    "AllReduce", mybir.AluOpType.add, replica_groups=rg,
                    ins=[stat_src.opt()], outs=[stat_dst.opt()])
                gstat = wp.tile([128, 2], f32, tag="st", name="gstat")
                nc.sync.dma_start(out=gstat[:], in_=stat_dst[:])
                mu = wp.tile([128, 1], f32, tag="st1", name="mu")
                nc.vector.tensor_scalar_mul(mu[:], gstat[:, 0:1], 1.0 / N_REAL)
                var = wp.tile([128, 1], f32, tag="st1", name="var")
                nc.vector.tensor_scalar_mul(var[:], gstat[:, 1:2], 1.0 / N_REAL)
                mu2 = wp.tile([128, 1], f32, tag="st1", name="mu2")
                nc.vector.tensor_tensor(out=mu2[:], in0=mu[:], in1=mu[:],
                                        op=mybir.AluOpType.mult)
                nc.vector.tensor_tensor(out=var[:], in0=var[:], in1=mu2[:],
                                        op=mybir.AluOpType.subtract)
                nc.vector.tensor_scalar_add(var[:], var[:], EPS)
                rinv = wp.tile([128, 1], f32, tag="st1", name="rinv")
                nc.vector.reciprocal(rinv[:], var[:])
                rs = wp.tile([128, 1], f32, tag="st1", name="rs")
                nc.scalar.sqrt(rs[:], rinv[:])
                gp = wp.tile([128, 1], f32, tag="st1", name="gp")
                nc.vector.tensor_tensor(out=gp[:], in0=bng[:, l:l + 1], in1=rs[:],
                                        op=mybir.AluOpType.mult)
                mgp = wp.tile([128, 1], f32, tag="st1", name="mgp")
                nc.vector.tensor_tensor(out=mgp[:], in0=mu[:], in1=gp[:],
                                        op=mybir.AluOpType.mult)
                bp = wp.tile([128, 1], f32, tag="st1", name="bp")
                nc.vector.tensor_tensor(out=bp[:], in0=bnb[:, l:l + 1], in1=mgp[:],
                                        op=mybir.AluOpType.subtract)

                # bn apply + relu (+ residual)
                for gs, w in groups:
                    if l == 0:
                        nc.scalar.activation(hT[:, gs:gs + w], convTb[:, gs:gs + w],
                                             mybir.ActivationFunctionType.Relu,
                                             bias=bp[:, 0:1], scale=gp[:, 0:1])
                    else:
                        hnew = wp.tile([128, 512], bf16, tag="hnew", name="hnew")
                        nc.scalar.activation(hnew[:, :w], convTb[:, gs:gs + w],
                                             mybir.ActivationFunctionType.Relu,
                                             bias=bp[:, 0:1], scale=gp[:, 0:1])
                        nc.vector.tensor_tensor(out=hT[:, gs:gs + w],
                                                in0=hT[:, gs:gs + w],
                                                in1=hnew[:, :w],
                                                op=mybir.AluOpType.add)

            # ---- classifier (fused per 512-col chunk) ----
            for gs, w in groups:
                nc.vector.tensor_copy(out=hpTb[:, gs:gs + w], in_=hT[:, gs:gs + w])
                ps1 = psp.tile([64, 512], f32, tag="mm", name="ps1")
                nc.tensor.matmul(out=ps1[:, :w], lhsT=wc1[:], rhs=hpTb[:, gs:gs + w],
                                 start=True, stop=True)
                h4 = wp.tile([64, 512], bf16, tag="h4", name="h4")
                nc.scalar.activation(h4[:, :w], ps1[:, :w],
                                     mybir.ActivationFunctionType.Relu,
                                     bias=bc1[:, 0:1], scale=1.0)
                ps2 = psp.tile([N_CLS, 512], f32, tag="mm", name="ps2")
                nc.tensor.matmul(out=ps2[:, :w], lhsT=wc2[:], rhs=h4[:, :w],
                                 start=True, stop=True)
                ot = wp.tile([N_CLS, 512], f32, tag="ot", name="ot")
                nc.vector.tensor_scalar(out=ot[:, :w], in0=ps2[:, :w],
                                        scalar1=bc2[:, 0:1], scalar2=None,
                                        op0=mybir.AluOpType.add)
                nc.sync.dma_start(out=out_d.ap()[:, gs:gs + w], in_=ot[:, :w])

    nc.compile()
    return nc


def make_host_inputs(inputs, P):
    x = np.asarray(inputs["x"], np.float32)
    core_nodes = P["core_nodes"]
    local_rows = P["local_rows"]
    dinv = P["dinv"]
    ident = np.eye(128, dtype=bf16_np)
    wconv = np.stack([np.asarray(inputs[f"W_conv{i}"], np.float32)
                      for i in range(3)]).astype(bf16_np)
    bng = np.stack([np.asarray(inputs[f"bn_g{i}"], np.float32)
                    for i in range(3)])[:, :, None]
    bnb = np.stack([np.asarray(inputs[f"bn_b{i}"], np.float32)
                    for i in range(3)])[:, :, None]
    maps = []
    for c in range(NC):
        xT = np.zeros((F_IN, LSHARD), np.float32)
        xT[:, local_rows] = x[core_nodes[c]].T
        dl = np.zeros((LSHARD,), np.float32)
        dl[local_rows] = dinv[core_nodes[c]]
        maps.append({
            "xT": xT,
            "idxA": P["idxA"][c],
            "idxB": P["idxB"][c],
            "dinvbc": np.broadcast_to(dl, (128, LSHARD)).copy(),
            "W_enc": np.asarray(inputs["W_enc"], np.float32),
            "b_enc": np.asarray(inputs["b_enc"], np.float32)[:, None],
            "W_conv": wconv,
            "bn_g": bng.astype(np.float32),
            "bn_b": bnb.astype(np.float32),
            "W_cls1": np.asarray(inputs["W_cls1"], np.float32).astype(bf16_np),
            "b_cls1": np.asarray(inputs["b_cls1"], np.float32)[:, None],
            "W_cls2": np.asarray(inputs["W_cls2"], np.float32).astype(bf16_np),
            "b_cls2": np.asarray(inputs["b_cls2"], np.float32)[:, None],
            "ident": ident,
        })
    return maps


def assemble_output(results, P):
    out = np.zeros((N_REAL, N_CLS), np.float32)
    for c in range(NC):
        out[P["core_nodes"][c]] = results[c]["outT"][:, P["local_rows"]].T
    return out


# ---------------- SPMD runner ----------------
class SpmdRunner:
    def __init__(self, nc, n_cores: int, donate: bool = True):
        install_neuronx_cc_hook()
        self.nc = nc
        self.n_cores = n_cores
        partition_name = nc.partition_id_tensor.name if nc.partition_id_tensor else None

        in_names: list[str] = []
        out_names: list[str] = []
        out_avals = []
        zero_outs: list[np.ndarray] = []
        for alloc in nc.m.functions[0].allocations:
            if not isinstance(alloc, mybir.MemoryLocationSet):
                continue
            name = alloc.memorylocations[0].name
            if alloc.kind == "ExternalInput":
                if name != partition_name:
                    in_names.append(name)
            elif alloc.kind == "ExternalOutput":
                shape = tuple(alloc.tensor_shape)
                dtype = mybir.dt.np(alloc.dtype)
                out_names.append(name)
                out_avals.append(jax.core.ShapedArray(shape, dtype))
                zero_outs.append(np.zeros(shape, dtype))
        self.in_names = in_names
        self.out_names = out_names
        self.out_avals = out_avals
        self.zero_outs = zero_outs
        n_params = len(in_names)
        n_outs = len(out_avals)
        all_names = list(in_names) + list(out_names)
        if partition_name is not None:
            all_names.append(partition_name)

        def _body(*args):
            operands = list(args)
            if partition_name is not None:
                operands.append(partition_id_tensor())
            outs = _bass_exec_p.bind(
                *operands,
                out_avals=tuple(out_avals),
                in_names=tuple(all_names),
                out_names=tuple(out_names),
                lowering_input_output_aliases=(),
                sim_require_finite=True,
                sim_require_nnan=True,
                nc=nc,
            )
            return tuple(outs)

        devices = jax.devices()[:n_cores]
        assert len(devices) == n_cores
        self.mesh = Mesh(np.asarray(devices), ("core",))
        in_specs = (PartitionSpec("core"),) * (n_params + n_outs)
        out_specs = (PartitionSpec("core"),) * n_outs
        donate_argnums = tuple(range(n_params, n_params + n_outs)) if donate else ()
        self.fn = jax.jit(
            shard_map(_body, mesh=self.mesh, in_specs=in_specs,
                      out_specs=out_specs, check_rep=False),
            donate_argnums=donate_argnums,
            keep_unused=True,
        )

    def concat_inputs(self, in_maps):
        n = self.n_cores
        return [
            np.concatenate([np.asarray(in_maps[c][name]) for c in range(n)], axis=0)
            for name in self.in_names
        ]

    def concat_zeros(self):
        return [np.zeros((self.n_cores * z.shape[0], *z.shape[1:]), z.dtype)
                for z in self.zero_outs]

    def run(self, in_maps):
        """Execute once; returns list (per core) of dicts name->np.ndarray."""
        concat_in = self.concat_inputs(in_maps)
        out_arrs = self.fn(*concat_in, *self.concat_zeros())
        res = []
        for c in range(self.n_cores):
            res.append({
                name: np.asarray(out_arrs[i]).reshape(
                    self.n_cores, *self.out_avals[i].shape)[c]
                for i, name in enumerate(self.out_names)
            })
        return res


_CACHE = {}


def kernel(**inputs):
    inputs = {k: np.asarray(v) for k, v in inputs.items()}
    P = preprocess(inputs["edge_index"])
    key = (tuple(int(d) for d in P["DbA"]), tuple(int(d) for d in P["DbB"]))
    if key not in _CACHE:
        nc = build(P)
        _CACHE[key] = SpmdRunner(nc, NC)
    r = _CACHE[key]
    in_maps = make_host_inputs(inputs, P)
    res = r.run(in_maps)
    return assemble_output(res, P)


# revision 20
# speedup vs baseline: 14.8704x; 13.0934x over previous
"""Self-contained 8-core Trainium2 Bass kernel for the BaseGNN problem.

kernel(**inputs) -> np.ndarray [50000, 72] float32.

v2 strategy: degree-sorted node sharding across 8 NeuronCores. Per conv
layer, h' = dinv*h is allgathered as a bf16 node-major table in DRAM
(two halves so row ids fit int16), edge messages are fetched with a few
large batched dma_gather calls (instead of ~1000 small indirect DMAs),
aggregated per 128-dst-node block on the TensorEngine into PSUM,
normalized (BN stats via AllReduce) and activated; encoder/classifier
matmuls are fused in, epilogues batched 512 columns at a time.
"""
import numpy as np
import ml_dtypes

import jax
from jax.sharding import Mesh, PartitionSpec
from jax.experimental.shard_map import shard_map

import concourse.bacc as bacc
import concourse.tile as tile
import concourse.mybir as mybir
from concourse import bass
from concourse.bass2jax import _bass_exec_p, install_neuronx_cc_hook, partition_id_tensor

N = 50000
E = 1000000
bf16_np = ml_dtypes.bfloat16

F_IN = 16
HID = 128
N_CLS = 72
EPS = 1e-5
NC = 8
PER_CORE = 6250
BLOCKS = 49
LSHARD = BLOCKS * 128
N_REAL = 50000

# --- v2 half-table layout (int16-safe row ids for dma_gather) ---
A_BLOCKS = 25
B_BLOCKS = BLOCKS - A_BLOCKS
HALF_A_ROWS = A_BLOCKS * 128       # 3200 local rows in half A
HALF_B_ROWS = B_BLOCKS * 128       # 3072 local rows in half B
NODES_A = HALF_A_ROWS - 1          # 3199 real nodes; local row 3199 is A's zero row
NODES_B = PER_CORE - NODES_A       # 3051 real nodes; local rows 6251.. are B zeros
TOT_A = NC * HALF_A_ROWS           # 25600 (< 32768)
TOT_B = NC * HALF_B_ROWS           # 24576 (< 32768)
ZROW_A = NODES_A                   # core 0's A pad row
ZROW_B = NODES_B                   # core 0's B pad row, rebased
CAP = 64                           # gather buffer capacity (chunks, per half)
GROUP = 4                          # blocks per epilogue group (512 cols)

f32 = mybir.dt.float32
bf16 = mybir.dt.bfloat16
i16 = mybir.dt.int16


# ---------------- host preprocessing ----------------
def preprocess(edge_index):
    src = np.asarray(edge_index[0], dtype=np.int64)
    dst = np.asarray(edge_index[1], dtype=np.int64)
    indeg = np.bincount(dst, minlength=N).astype(np.int64)
    deg = (indeg + 1).astype(np.float32)
    dinv = (1.0 / np.sqrt(deg)).astype(np.float32)

    # half membership: global top NC*NODES_A nodes by indeg -> A
    order = np.argsort(-indeg, kind="stable")
    isA = np.zeros(N, dtype=bool)
    isA[order[:NC * NODES_A]] = True
    # per-dst counts of in-edges by source half
    dA = np.bincount(dst[isA[src]], minlength=N).astype(np.int64)
    dB = indeg - dA

    # snake-deal nodes in (indeg desc, A-deg desc) order -> 8 cores x 6250;
    # position j of every core holds global ranks 8j..8j+7, so per-core
    # degree profiles align rank-by-rank (tight shared block maxima) and
    # the first NODES_A positions of each core are exactly the A half.
    # boustrophedon over (indeg, dA): alternate dA direction per degree run so
    # dA stays continuous across run boundaries (tight per-block maxima for
    # both halves even when a block straddles a degree boundary)
    dA_snake = np.where(indeg % 2 == 0, -dA, dA)
    ord2 = np.lexsort((dA_snake, (~isA).astype(np.int64), -indeg))
    rounds = PER_CORE // 2
    assert rounds * 2 * NC == N
    core_nodes = np.empty((NC, PER_CORE), dtype=np.int64)
    fwd = ord2.reshape(rounds * 2, NC)
    core_nodes[:, 0::2] = fwd[0::2].T
    core_nodes[:, 1::2] = fwd[1::2][:, ::-1].T
    assert isA[core_nodes[:, :NODES_A]].all()
    assert not isA[core_nodes[:, NODES_A:]].any()

    # local rows: positions 0..NODES_A-1 -> rows 0..3198; rest -> 3200..6250
    local_rows = np.concatenate([np.arange(NODES_A),
                                 HALF_A_ROWS + np.arange(NODES_B)])
    core_of = np.empty(N, dtype=np.int64)
    lrow_of = np.empty(N, dtype=np.int64)
    for c in range(NC):
        core_of[core_nodes[c]] = c
        lrow_of[core_nodes[c]] = local_rows

    # int16 table rows per half
    rowh = np.empty(N, dtype=np.int64)
    a_mask_node = lrow_of < HALF_A_ROWS
    rowh[a_mask_node] = core_of[a_mask_node] * HALF_A_ROWS + lrow_of[a_mask_node]
    bm = ~a_mask_node
    rowh[bm] = core_of[bm] * HALF_B_ROWS + (lrow_of[bm] - HALF_A_ROWS)

    # per-(core, block) chunk counts shared across cores
    dA_pad = np.zeros((NC, LSHARD), dtype=np.int64)
    dB_pad = np.zeros((NC, LSHARD), dtype=np.int64)
    for c in range(NC):
        dA_pad[c, local_rows] = dA[core_nodes[c]]
        dB_pad[c, local_rows] = dB[core_nodes[c]]
    DbA = dA_pad.reshape(NC, BLOCKS, 128).max(axis=(0, 2))
    DbB = dB_pad.reshape(NC, BLOCKS, 128).max(axis=(0, 2))
    both_zero = (DbA + DbB) == 0
    DbB[both_zero] = 1
    cbA = np.concatenate([[0], np.cumsum(DbA)]).astype(np.int64)
    cbB = np.concatenate([[0], np.cumsum(DbB)]).astype(np.int64)
    nchA = int(cbA[-1])
    nchB = int(cbB[-1])

    # vectorized slot-stream construction
    blk_of = lrow_of // 128
    p_of = lrow_of % 128

    def build_stream(sel_mask, cb, nch, zrow, half_is_a):
        e_sel = np.nonzero(sel_mask)[0]
        d_sel = dst[e_sel]
        o = np.argsort(d_sel, kind="stable")
        e_sel = e_sel[o]
        d_sel = d_sel[o]
        s_sel = src[e_sel]
        # rank of each edge within its dst
        starts = np.searchsorted(d_sel, np.arange(N))
        rank = np.arange(len(e_sel)) - starts[d_sel]
        slot = (cb[blk_of[d_sel]] + rank) * 128 + p_of[d_sel]
        stream = np.full((NC, nch * 128), zrow, dtype=np.int16)
        val = rowh[s_sel]
        if half_is_a:
            assert (val < TOT_A).all()
        else:
            assert (val < TOT_B).all()
        stream[core_of[d_sel], slot] = val.astype(np.int16)
        return stream

    streamA = build_stream(isA[src], cbA, nchA, ZROW_A, True)
    streamB = build_stream(~isA[src], cbB, nchB, ZROW_B, False)

    def wrap16(stream):
        # [NC, nch*128] -> [NC, 128, nch*8]; value for position j at
        # (j % 16, j // 16), replicated across the 8 groups of 16 partitions
        ncol = stream.shape[1] // 16
        w = stream.reshape(NC, ncol, 16).transpose(0, 2, 1)  # [NC, 16, ncol]
        return np.tile(w, (1, 8, 1)).copy()

    idxA = wrap16(streamA)
    idxB = wrap16(streamB)

    # slabs: greedy ranges of blocks with per-half chunk counts <= CAP
    slabs = []
    b0 = 0
    while b0 < BLOCKS:
        b1 = b0 + 1
        while (b1 < BLOCKS
               and cbA[b1 + 1] - cbA[b0] <= CAP
               and cbB[b1 + 1] - cbB[b0] <= CAP):
            b1 += 1
        assert cbA[b1] - cbA[b0] <= CAP and cbB[b1] - cbB[b0] <= CAP
        slabs.append((b0, b1, int(cbA[b0]), int(cbA[b1]),
                      int(cbB[b0]), int(cbB[b1])))
        b0 = b1

    waste = (nchA + nchB) * 128 / (E / NC)
    return dict(dinv=dinv, core_nodes=core_nodes, local_rows=local_rows,
                DbA=DbA, DbB=DbB, cbA=cbA, cbB=cbB, nchA=nchA, nchB=nchB,
                idxA=idxA, idxB=idxB, slabs=slabs, waste=waste)


# ---------------- kernel build ----------------
def build(P, skip_gather=False, skip_coll=False):
    cbA, cbB = P["cbA"], P["cbB"]
    nchA, nchB = P["nchA"], P["nchB"]
    slabs = P["slabs"]
    SA = nchA * 8
    SB = nchB * 8
    nslab = len(slabs)

    nc = bacc.Bacc("TRN2", target_bir_lowering=False, debug=False,
                   enable_asserts=True, num_devices=NC)

    # ---- inputs ----
    xT_d = nc.dram_tensor("xT", [F_IN, LSHARD], f32, kind="ExternalInput")
    idxA_d = nc.dram_tensor("idxA", [128, SA], i16, kind="ExternalInput")
    idxB_d = nc.dram_tensor("idxB", [128, SB], i16, kind="ExternalInput")
    dinv_d = nc.dram_tensor("dinvbc", [128, LSHARD], f32, kind="ExternalInput")
    wenc_d = nc.dram_tensor("W_enc", [F_IN, HID], f32, kind="ExternalInput")
    benc_d = nc.dram_tensor("b_enc", [HID, 1], f32, kind="ExternalInput")
    wc_d = nc.dram_tensor("W_conv", [3, HID, HID], bf16, kind="ExternalInput")
    bng_d = nc.dram_tensor("bn_g", [3, HID, 1], f32, kind="ExternalInput")
    bnb_d = nc.dram_tensor("bn_b", [3, HID, 1], f32, kind="ExternalInput")
    wc1_d = nc.dram_tensor("W_cls1", [HID, 64], bf16, kind="ExternalInput")
    bc1_d = nc.dram_tensor("b_cls1", [64, 1], f32, kind="ExternalInput")
    wc2_d = nc.dram_tensor("W_cls2", [64, N_CLS], bf16, kind="ExternalInput")
    bc2_d = nc.dram_tensor("b_cls2", [N_CLS, 1], f32, kind="ExternalInput")
    ident_d = nc.dram_tensor("ident", [128, 128], bf16, kind="ExternalInput")
    out_d = nc.dram_tensor("outT", [N_CLS, LSHARD], f32, kind="ExternalOutput")

    rg = [list(range(NC))]

    # column groups for 512-wide epilogues: 12 x 512 + 1 x 128
    groups = []
    s = 0
    while s < LSHARD:
        w = min(GROUP * 128, LSHARD - s)
        groups.append((s, w))
        s += w

    with tile.TileContext(nc) as tc:
        with tc.tile_pool(name="persist", bufs=1) as pp, \
             tc.tile_pool(name="work", bufs=4) as wp, \
             tc.tile_pool(name="psum", bufs=2, space="PSUM") as psp, \
             tc.tile_pool(name="dram", bufs=1, space="DRAM") as dp:

            # ---- persistent SBUF ----
            hT = pp.tile([128, LSHARD], f32, name="hT")
            hpTb = pp.tile([128, LSHARD], bf16, name="hpTb")
            convTb = pp.tile([128, LSHARD], bf16, name="convTb")
            dinv = pp.tile([128, LSHARD], f32, name="dinv")
            idxA_sb = pp.tile([128, SA], i16, name="idxA_sb")
            idxB_sb = pp.tile([128, SB], i16, name="idxB_sb")
            identb = pp.tile([128, 128], bf16, name="identb")
            wenc = pp.tile([F_IN, HID], f32, name="wenc")
            benc = pp.tile([HID, 1], f32, name="benc")
            wc = [pp.tile([HID, HID], bf16, name=f"wc{i}") for i in range(3)]
            bng = pp.tile([HID, 3], f32, name="bng")
            bnb = pp.tile([HID, 3], f32, name="bnb")
            wc1 = pp.tile([HID, 64], bf16, name="wc1")
            bc1 = pp.tile([64, 1], f32, name="bc1")
            wc2 = pp.tile([64, N_CLS], bf16, name="wc2")
            bc2 = pp.tile([N_CLS, 1], f32, name="bc2")
            bnst = pp.tile([128, len(groups) * 6], f32, name="bnst")

            nc.sync.dma_start(out=dinv[:], in_=dinv_d.ap())
            nc.sync.dma_start(out=idxA_sb[:], in_=idxA_d.ap())
            nc.sync.dma_start(out=idxB_sb[:], in_=idxB_d.ap())
            nc.sync.dma_start(out=identb[:], in_=ident_d.ap())
            nc.sync.dma_start(out=wenc[:], in_=wenc_d.ap())
            nc.sync.dma_start(out=benc[:], in_=benc_d.ap())
            for l in range(3):
                nc.sync.dma_start(out=wc[l][:], in_=wc_d.ap()[l])
                nc.sync.dma_start(out=bng[:, l:l + 1], in_=bng_d.ap()[l])
                nc.sync.dma_start(out=bnb[:, l:l + 1], in_=bnb_d.ap()[l])
            nc.sync.dma_start(out=wc1[:], in_=wc1_d.ap())
            nc.sync.dma_start(out=bc1[:], in_=bc1_d.ap())
            nc.sync.dma_start(out=wc2[:], in_=wc2_d.ap())
            nc.sync.dma_start(out=bc2[:], in_=bc2_d.ap())

            # ---- encoder: hT = relu(Wenc^T @ xT + b), xT streamed per chunk ----
            for gs, w in groups:
                xch = wp.tile([F_IN, 512], f32, tag="xch", name="xch")
                nc.sync.dma_start(out=xch[:, :w], in_=xT_d.ap()[:, gs:gs + w])
                pse = psp.tile([128, 512], f32, tag="mm", name="pse")
                nc.tensor.matmul(out=pse[:, :w], lhsT=wenc[:],
                                 rhs=xch[:, :w], start=True, stop=True)
                nc.scalar.activation(hT[:, gs:gs + w], pse[:, :w],
                                     mybir.ActivationFunctionType.Relu,
                                     bias=benc[:, 0:1], scale=1.0)

            # ---- conv layers ----
            for l in range(3):
                bounceA = dp.tile([HALF_A_ROWS, HID], bf16, name=f"bounceA{l}")
                bounceB = dp.tile([HALF_B_ROWS, HID], bf16, name=f"bounceB{l}")
                tadr = "Local" if skip_coll else "Shared"
                tableA = dp.tile([TOT_A, HID], bf16, addr_space=tadr,
                                 name=f"tableA{l}")
                tableB = dp.tile([TOT_B, HID], bf16, addr_space=tadr,
                                 name=f"tableB{l}")

                # h' = hT * dinv -> bf16; transpose to node-major; bounce
                for gi, (gs, w) in enumerate(groups):
                    nc.vector.tensor_tensor(out=hpTb[:, gs:gs + w],
                                            in0=hT[:, gs:gs + w],
                                            in1=dinv[:, gs:gs + w],
                                            op=mybir.AluOpType.mult)
                    pst = psp.tile([128, 512], f32, tag="tr", name="pst")
                    nblk = w // 128
                    for j in range(nblk):
                        nc.tensor.matmul(out=pst[:, j * 128:(j + 1) * 128],
                                         lhsT=hpTb[:, gs + j * 128:gs + (j + 1) * 128],
                                         rhs=identb[:], start=True, stop=True)
                    trs = wp.tile([128, 512], bf16, tag="trs", name="trs")
                    nc.scalar.copy(out=trs[:, :w], in_=pst[:, :w])
                    for j in range(nblk):
                        b = gi * GROUP + j
                        bs = b * 128
                        if b < A_BLOCKS:
                            nc.sync.dma_start(
                                out=bounceA[bs:bs + 128, :],
                                in_=trs[:, j * 128:(j + 1) * 128])
                        else:
                            bs2 = (b - A_BLOCKS) * 128
                            nc.sync.dma_start(
                                out=bounceB[bs2:bs2 + 128, :],
                                in_=trs[:, j * 128:(j + 1) * 128])

                if skip_coll:
                    for cc in range(NC):
                        nc.sync.dma_start(
                            out=tableA[cc * HALF_A_ROWS:(cc + 1) * HALF_A_ROWS, :],
                            in_=bounceA[:])
                        nc.sync.dma_start(
                            out=tableB[cc * HALF_B_ROWS:(cc + 1) * HALF_B_ROWS, :],
                            in_=bounceB[:])
                else:
                    nc.gpsimd.collective_compute(
                        "AllGather", mybir.AluOpType.bypass, replica_groups=rg,
                        ins=[bounceA.opt()], outs=[tableA.opt()])
                    nc.gpsimd.collective_compute(
                        "AllGather", mybir.AluOpType.bypass, replica_groups=rg,
                        ins=[bounceB.opt()], outs=[tableB.opt()])

                # batched gathers per slab
                gA_tiles = [None] * nslab
                gB_tiles = [None] * nslab
                for si, (b0, b1, a0, a1, bb0, bb1) in enumerate(slabs):
                    if skip_gather:
                        continue
                    na = a1 - a0
                    nb = bb1 - bb0
                    if na > 0:
                        gA = wp.tile([128, CAP, 128], bf16, tag="gA",
                                     bufs=2, name=f"gA{l}_{si}")
                        nc.gpsimd.dma_gather(
                            out_ap=gA[:, :na, :], in_ap=tableA[:],
                            idxs_ap=idxA_sb[:, a0 * 8:a1 * 8],
                            num_idxs=na * 128, num_idxs_reg=na * 128,
                            elem_size=HID, single_packet=False)
                        gA_tiles[si] = gA
                    if nb > 0:
                        gB = wp.tile([128, CAP, 128], bf16, tag="gB",
                                     bufs=2, name=f"gB{l}_{si}")
                        nc.gpsimd.dma_gather(
                            out_ap=gB[:, :nb, :], in_ap=tableB[:],
                            idxs_ap=idxB_sb[:, bb0 * 8:bb1 * 8],
                            num_idxs=nb * 128, num_idxs_reg=nb * 128,
                            elem_size=HID, single_packet=False)
                        gB_tiles[si] = gB

                blk_slab = np.empty(BLOCKS, dtype=np.int64)
                for si, (b0, b1, *_rest) in enumerate(slabs):
                    blk_slab[b0:b1] = si

                # per-group aggregation + conv + stats
                for gi, (gs, w) in enumerate(groups):
                    nblk = w // 128
                    psa = psp.tile([128, 512], f32, tag="agg", name="psa")
                    for j in range(nblk):
                        b = gi * GROUP + j
                        si = int(blk_slab[b])
                        _, _, a0, _, bb0, _ = slabs[si]
                        if skip_gather:
                            nc.tensor.matmul(
                                out=psa[:, j * 128:(j + 1) * 128],
                                lhsT=hpTb[:, bs2:bs2 + 128] if False else hpTb[:, (gi * GROUP + j) * 128:(gi * GROUP + j + 1) * 128],
                                rhs=identb[:], start=True, stop=True)
                            continue
                        seq = []
                        for c in range(int(cbA[b]), int(cbA[b + 1])):
                            seq.append((gA_tiles[si], c - a0))
                        for c in range(int(cbB[b]), int(cbB[b + 1])):
                            seq.append((gB_tiles[si], c - bb0))
                        assert seq, f"block {b} has no chunks"
                        for k, (gt, off) in enumerate(seq):
                            nc.tensor.matmul(
                                out=psa[:, j * 128:(j + 1) * 128],
                                lhsT=gt[:, off, :], rhs=identb[:],
                                start=(k == 0), stop=(k == len(seq) - 1))
                    at = wp.tile([128, 512], bf16, tag="at", name="at")
                    nc.vector.tensor_tensor(out=at[:, :w], in0=psa[:, :w],
                                            in1=hpTb[:, gs:gs + w],
                                            op=mybir.AluOpType.add)
                    psc = psp.tile([128, 512], f32, tag="mm", name="psc")
                    nc.tensor.matmul(out=psc[:, :w], lhsT=wc[l][:],
                                     rhs=at[:, :w], start=True, stop=True)
                    nc.vector.tensor_tensor(out=convTb[:, gs:gs + w],
                                            in0=psc[:, :w],
                                            in1=dinv[:, gs:gs + w],
                                            op=mybir.AluOpType.mult)
                    nc.vector.bn_stats(out=bnst[:, gi * 6:(gi + 1) * 6],
                                       in_=convTb[:, gs:gs + w])

                # global BN stats
                bnagg = wp.tile([128, 2], f32, tag="st", name="bnagg")
                nc.vector.bn_aggr(out=bnagg[:], in_=bnst[:])
                ssum = wp.tile([128, 2], f32, tag="st", name="ssum")
                m2 = wp.tile([128, 1], f32, tag="st1", name="m2")
                nc.vector.tensor_tensor(out=m2[:], in0=bnagg[:, 0:1],
                                        in1=bnagg[:, 0:1], op=mybir.AluOpType.mult)
                nc.vector.tensor_scalar_mul(ssum[:, 0:1], bnagg[:, 0:1],
                                            float(LSHARD))
                q = wp.tile([128, 1], f32, tag="st1", name="q")
                nc.vector.tensor_tensor(out=q[:], in0=bnagg[:, 1:2], in1=m2[:],
                                        op=mybir.AluOpType.add)
                nc.vector.tensor_scalar_mul(ssum[:, 1:2], q[:], float(LSHARD))
                gstat = wp.tile([128, 2], f32, tag="st", name="gstat")
                if skip_coll:
                    nc.vector.tensor_scalar_mul(gstat[:], ssum[:], float(NC))
                else:
                    stat_src = dp.tile([128, 2], f32, name=f"stat_src{l}")
                    stat_dst = dp.tile([128, 2], f32, addr_space="Shared",
                                       name=f"stat_dst{l}")
                    nc.sync.dma_start(out=stat_src[:], in_=ssum[:])
                    nc.gpsimd.collective_compute(
                        "AllReduce", mybir.AluOpType.add, replica_groups=rg,
                        ins=[stat_src.opt()], outs=[stat_dst.opt()])
                    nc.sync.dma_start(out=gstat[:], in_=stat_dst[:])
                mu = wp.tile([128, 1], f32, tag="st1", name="mu")
                nc.vector.tensor_scalar_mul(mu[:], gstat[:, 0:1], 1.0 / N_REAL)
                var = wp.tile([128, 1], f32, tag="st1", name="var")
                nc.vector.tensor_scalar_mul(var[:], gstat[:, 1:2], 1.0 / N_REAL)
                mu2 = wp.tile([128, 1], f32, tag="st1", name="mu2")
                nc.vector.tensor_tensor(out=mu2[:], in0=mu[:], in1=mu[:],
                                        op=mybir.AluOpType.mult)
                nc.vector.tensor_tensor(out=var[:], in0=var[:], in1=mu2[:],
                                        op=mybir.AluOpType.subtract)
                nc.vector.tensor_scalar_add(var[:], var[:], EPS)
                rinv = wp.tile([128, 1], f32, tag="st1", name="rinv")
                nc.vector.reciprocal(rinv[:], var[:])
                rs = wp.tile([128, 1], f32, tag="st1", name="rs")
                nc.scalar.sqrt(rs[:], rinv[:])
                gp = wp.tile([128, 1], f32, tag="st1", name="gp")
                nc.vector.tensor_tensor(out=gp[:], in0=bng[:, l:l + 1], in1=rs[:],
                                        op=mybir.AluOpType.mult)
                mgp = wp.tile([128, 1], f32, tag="st1", name="mgp")
                nc.vector.tensor_tensor(out=mgp[:], in0=mu[:], in1=gp[:],
                                        op=mybir.AluOpType.mult)
                bp = wp.tile([128, 1], f32, tag="st1", name="bp")
                nc.vector.tensor_tensor(out=bp[:], in0=bnb[:, l:l + 1], in1=mgp[:],
                                        op=mybir.AluOpType.subtract)

                # bn apply + relu (+ residual)
                for gs, w in groups:
                    if l == 0:
                        nc.scalar.activation(hT[:, gs:gs + w], convTb[:, gs:gs + w],
                                             mybir.ActivationFunctionType.Relu,
                                             bias=bp[:, 0:1], scale=gp[:, 0:1])
                    else:
                        hnew = wp.tile([128, 512], bf16, tag="hnew", name="hnew")
                        nc.scalar.activation(hnew[:, :w], convTb[:, gs:gs + w],
                                             mybir.ActivationFunctionType.Relu,
                                             bias=bp[:, 0:1], scale=gp[:, 0:1])
                        nc.vector.tensor_tensor(out=hT[:, gs:gs + w],
                                                in0=hT[:, gs:gs + w],
                                                in1=hnew[:, :w],
                                                op=mybir.AluOpType.add)

            # ---- classifier (fused per 512-col chunk) ----
            for gs, w in groups:
                nc.vector.tensor_copy(out=hpTb[:, gs:gs + w], in_=hT[:, gs:gs + w])
                ps1 = psp.tile([64, 512], f32, tag="mm", name="ps1")
                nc.tensor.matmul(out=ps1[:, :w], lhsT=wc1[:], rhs=hpTb[:, gs:gs + w],
                                 start=True, stop=True)
                h4 = wp.tile([64, 512], bf16, tag="h4", name="h4")
                nc.scalar.activation(h4[:, :w], ps1[:, :w],
                                     mybir.ActivationFunctionType.Relu,
                                     bias=bc1[:, 0:1], scale=1.0)
                ps2 = psp.tile([N_CLS, 512], f32, tag="mm", name="ps2")
                nc.tensor.matmul(out=ps2[:, :w], lhsT=wc2[:], rhs=h4[:, :w],
                                 start=True, stop=True)
                ot = wp.tile([N_CLS, 512], f32, tag="ot", name="ot")
                nc.vector.tensor_scalar(out=ot[:, :w], in0=ps2[:, :w],
                                        scalar1=bc2[:, 0:1], scalar2=None,
                                        op0=mybir.AluOpType.add)
                nc.sync.dma_start(out=out_d.ap()[:, gs:gs + w], in_=ot[:, :w])

    nc.compile()
    return nc


def make_host_inputs(inputs, P):
    x = np.asarray(inputs["x"], np.float32)
    core_nodes = P["core_nodes"]
    local_rows = P["local_rows"]
    dinv = P["dinv"]
    ident = np.eye(128, dtype=bf16_np)
    wconv = np.stack([np.asarray(inputs[f"W_conv{i}"], np.float32)
                      for i in range(3)]).astype(bf16_np)
    bng = np.stack([np.asarray(inputs[f"bn_g{i}"], np.float32)
                    for i in range(3)])[:, :, None]
    bnb = np.stack([np.asarray(inputs[f"bn_b{i}"], np.float32)
                    for i in range(3)])[:, :, None]
    maps = []
    for c in range(NC):
        xT = np.zeros((F_IN, LSHARD), np.float32)
        xT[:, local_rows] = x[core_nodes[c]].T
        dl = np.zeros((LSHARD,), np.float32)
        dl[local_rows] = dinv[core_nodes[c]]
        maps.append({
            "xT": xT,
            "idxA": P["idxA"][c],
            "idxB": P["idxB"][c],
            "dinvbc": np.broadcast_to(dl, (128, LSHARD)).copy(),
            "W_enc": np.asarray(inputs["W_enc"], np.float32),
            "b_enc": np.asarray(inputs["b_enc"], np.float32)[:, None],
            "W_conv": wconv,
            "bn_g": bng.astype(np.float32),
            "bn_b": bnb.astype(np.float32),
            "W_cls1": np.asarray(inputs["W_cls1"], np.float32).astype(bf16_np),
            "b_cls1": np.asarray(inputs["b_cls1"], np.float32)[:, None],
            "W_cls2": np.asarray(inputs["W_cls2"], np.float32).astype(bf16_np),
            "b_cls2": np.asarray(inputs["b_cls2"], np.float32)[:, None],
            "ident": ident,
        })
    return maps


def assemble_output(results, P):
    out = np.zeros((N_REAL, N_CLS), np.float32)
    for c in range(NC):
        out[P["core_nodes"][c]] = results[c]["outT"][:, P["local_rows"]].T
    return out


# ---------------- SPMD runner ----------------
class SpmdRunner:
    def __init__(self, nc, n_cores: int, donate: bool = True):
        install_neuronx_cc_hook()
        self.nc = nc
        self.n_cores = n_cores
        partition_name = nc.partition_id_tensor.name if nc.partition_id_tensor else None

        in_names: list[str] = []
        out_names: list[str] = []
        out_avals = []
        zero_outs: list[np.ndarray] = []
        for alloc in nc.m.functions[0].allocations:
            if not isinstance(alloc, mybir.MemoryLocationSet):
                continue
            name = alloc.memorylocations[0].name
            if alloc.kind == "ExternalInput":
                if name != partition_name:
                    in_names.append(name)
            elif alloc.kind == "ExternalOutput":
                shape = tuple(alloc.tensor_shape)
                dtype = mybir.dt.np(alloc.dtype)
                out_names.append(name)
                out_avals.append(jax.core.ShapedArray(shape, dtype))
                zero_outs.append(np.zeros(shape, dtype))
        self.in_names = in_names
        self.out_names = out_names
        self.out_avals = out_avals
        self.zero_outs = zero_outs
        n_params = len(in_names)
        n_outs = len(out_avals)
        all_names = list(in_names) + list(out_names)
        if partition_name is not None:
            all_names.append(partition_name)

        def _body(*args):
            operands = list(args)
            if partition_name is not None:
                operands.append(partition_id_tensor())
            outs = _bass_exec_p.bind(
                *operands,
                out_avals=tuple(out_avals),
                in_names=tuple(all_names),
                out_names=tuple(out_names),
                lowering_input_output_aliases=(),
                sim_require_finite=True,
                sim_require_nnan=True,
                nc=nc,
            )
            return tuple(outs)

        devices = jax.devices()[:n_cores]
        assert len(devices) == n_cores
        self.mesh = Mesh(np.asarray(devices), ("core",))
        self.sharding = jax.sharding.NamedSharding(self.mesh, PartitionSpec("core"))
        in_specs = (PartitionSpec("core"),) * (n_params + n_outs)
        out_specs = (PartitionSpec("core"),) * n_outs
        donate_argnums = tuple(range(n_params, n_params + n_outs)) if donate else ()
        self.fn = jax.jit(
            shard_map(_body, mesh=self.mesh, in_specs=in_specs,
                      out_specs=out_specs, check_rep=False),
            donate_argnums=donate_argnums,
            keep_unused=True,
        )

    def concat_inputs(self, in_maps):
        n = self.n_cores
        return [
            np.concatenate([np.asarray(in_maps[c][name]) for c in range(n)], axis=0)
            for name in self.in_names
        ]

    def concat_zeros(self):
        return [np.zeros((self.n_cores * z.shape[0], *z.shape[1:]), z.dtype)
                for z in self.zero_outs]

    def run(self, in_maps):
        """Execute once; returns list (per core) of dicts name->np.ndarray."""
        concat_in = [jax.device_put(v, self.sharding)
                     for v in self.concat_inputs(in_maps)]
        zeros = [jax.device_put(z, self.sharding) for z in self.concat_zeros()]
        out_arrs = self.fn(*concat_in, *zeros)
        res = []
        for c in range(self.n_cores):
            res.append({
                name: np.asarray(out_arrs[i]).reshape(
                    self.n_cores, *self.out_avals[i].shape)[c]
                for i, name in enumerate(self.out_names)
            })
        return res


_CACHE = {}


def kernel(**inputs):
    inputs = {k: np.asarray(v) for k, v in inputs.items()}
    P = preprocess(inputs["edge_index"])
    key = (tuple(int(d) for d in P["DbA"]), tuple(int(d) for d in P["DbB"]))
    if key not in _CACHE:
        nc = build(P)
        _CACHE[key] = SpmdRunner(nc, NC)
    r = _CACHE[key]
    in_maps = make_host_inputs(inputs, P)
    res = r.run(in_maps)
    return assemble_output(res, P)
